# revision 86
# baseline (speedup 1.0000x reference)
"""BPaCo+ loss on 8 TRN2 NeuronCores — v6 (fp8 DoubleRow pass-2).

Column-sharded softmax denominators: each core computes, for all B=1024
anchors, the partial sums of its 4224-column shard of the [B, C+B+K]
masked softmax (branch 2) and of the [B, B+C] softmax (branch 1):

  pass1 (PE):  P[j,i] = f_j . f_i / T            [128 j, 1024 i] per slot
  exp -> fp8e5:
      ACT lane: exp table, output e5m2 scaled 2^-5 (diag-bearing slots
                live here: e^14.29 * 2^-5 stays finite in e5m2)
      DVE lane: Schraudolph bit-trick straight into e5m2 codes via an
                int8 tensor_scalar (scale 2^-4), mean-calibrated
  pass2 (PE):  DoubleRow fp8 matmuls contract 256 j per instruction:
               acc[0,i]   += sum_j E[j,i] r0_j      (per-row fp8 scales)
               acc[1+c,i] += sum_j E[j,i] d_j [lab_j==c]
  select:      S_i = acc[0,i] + acc[1+lab_i,i]  (ohb mask + per-chunk
               transpose matmuls -> partS [128,16])

Diagonal masks multiply the fp8 E tiles on the GpSimd engine (otherwise
idle). Branch 1 (one DoubleRow pair into acc1) is exp'd first; its DR,
select and the sup-logits/single-slot matmuls are emitted a few pairs
into the main loop so the greedy per-engine scheduler keeps the
pass1->exp->DR pipeline fed (the in-order PE otherwise commits to a
LDWEIGHTS whose matmul input is not ready). DR matmuls trail their
exps by four pairs for the same reason; because of that lag the acc2
accumulation group is OPENED by the sup-logits matmuls (start=True
there), which are the first acc2 matmuls in emission order — moving
them without moving the start flag silently wipes their contribution.
Warm-up exp/matmuls overlap the
ACT table load and the PE p-state ramp with the input DMA head; DMAs
are ordered by first consumption. The sup-logits block needs no exp on
device: exp(sup) is precomputed on host and fed as a bf16 pass-2 rhs.
The mask-weighted logit sums A_i are O(B*D) and computed on host, as is
the final log/mean + cross-core reduction.
"""
import numpy as np
import ml_dtypes

from concourse import bass, bacc, mybir, tile
from concourse.bass_utils import run_bass_kernel_spmd

B, K, C, D = 1024, 32768, 100, 128
N = B + K
T, ALPHA = 0.07, 0.05
M = 8                      # cores
SHARD = N // M             # 4224 b2 columns per core
JC_B2 = SHARD // 128       # 33 main slots
BF16 = mybir.dt.bfloat16
F32 = mybir.dt.float32
FP8E4 = mybir.dt.float8e4
FP8E5 = mybir.dt.float8e5
I8 = mybir.dt.int8
NP_BF16 = ml_dtypes.bfloat16
NP_FP8E4 = ml_dtypes.float8_e4m3fn
NP_FP8E5 = ml_dtypes.float8_e5m2

# DVE-lane Schraudolph e5m2 exp (scale 2^-2): codes = trunc(x*4/ln2 + B5).
# The +8 octave shift (vs the 2^-4 calibration) keeps the code-0 underflow
# cliff at logit -9.06 (-7.2 sigma), far below any realistic dot product.
SCH_A5 = 4.0 / float(np.log(2.0))
SCH_B5 = 44.28 - 0.5 + 8.0   # -0.5: the HW DVE f32->int8 convert rounds
# ACT-lane exp bias: scale 2^-5, +0.003 cancels the e5m2 RNE bias
ACT_BIAS = -5.0 * float(np.log(2.0)) + 0.003

# lanes: ACT = diag-bearing slots {0..7, 33} + fillers; DVE = the rest
ACT_SLOTS = set(range(8)) | {9, 11, 13, 15, 17, 19, 21, 23, 32, 33, 34}
# acc2 DoubleRow pairs (one DVE + one ACT slot each) and the leftover single
PAIRS = [(8, 0), (10, 1), (12, 2), (14, 3), (16, 4), (18, 5), (20, 6),
         (22, 7), (24, 9), (25, 11), (26, 13), (27, 15), (28, 17), (29, 19),
         (30, 21), (31, 23)]
SINGLE = 32                # ACT lane, plain fp8 matmuls
ACC1_PAIR = (33, 34)       # (ACT b1-batch chunk, DVE centers)
# processing order (defines fTs column layout)
PROC = [ACC1_PAIR[0], ACC1_PAIR[1], SINGLE] + [s for p in PAIRS for s in p]
N_SLOT = 35
NPAIR = len(PAIRS) + 1     # + acc1 pair (block 0)

_CACHE = {}


def _build_nc():
    nc = bacc.Bacc(None, target_bir_lowering=False)
    fbT = nc.declare_dram_parameter("fbT", [D, B], BF16, isOutput=False)
    fTs = nc.declare_dram_parameter("fTs", [D, N_SLOT * 128], BF16, isOutput=False)
    Wp = nc.declare_dram_parameter("Wp", [128, NPAIR * 224], FP8E4, isOutput=False)
    W32 = nc.declare_dram_parameter("W32", [128, 101], FP8E4, isOutput=False)
    Wsup = nc.declare_dram_parameter("Wsup", [128, 101], BF16, isOutput=False)
    antidiag = nc.declare_dram_parameter("antidiag", [128, 128], BF16, isOutput=False)
    m33 = nc.declare_dram_parameter("m33", [128, B], BF16, isOutput=False)
    ohb1 = nc.declare_dram_parameter("ohb1", [128, B], BF16, isOutput=False)
    ohb2 = nc.declare_dram_parameter("ohb2", [128, B], BF16, isOutput=False)
    esup = nc.declare_dram_parameter("esup", [128, B], BF16, isOutput=False)
    partS = nc.declare_dram_parameter("partS", [128, 16], F32, isOutput=True)

    pos_of = {s: i for i, s in enumerate(PROC)}   # slot id -> fTs block

    with tile.TileContext(nc) as tc:
        with (
            tc.tile_pool(name="const", bufs=1) as cpool,
            tc.tile_pool(name="epool", bufs=20) as epool,
            tc.tile_pool(name="small", bufs=2) as spool,
            tc.tile_pool(name="mainps", bufs=3, space=bass.MemorySpace.PSUM) as mps,
            tc.tile_pool(name="accps", bufs=1, space=bass.MemorySpace.PSUM) as aps,
        ):
            # ---- warm-up + resident inputs (DMA in consumption order) ----
            warm_sb = cpool.tile([128, 512], BF16, tag="warm")
            nc.gpsimd.memset(warm_sb[:], 1.0)
            ones_sb = cpool.tile([128, 1], F32, tag="ones")
            nc.gpsimd.memset(ones_sb[:], 1.0)
            bias_sb = cpool.tile([128, 1], F32, tag="bias")
            nc.gpsimd.memset(bias_sb[:], ACT_BIAS)
            wexp_sb = cpool.tile([128, 1], BF16, tag="wexp")
            nc.scalar.activation(wexp_sb[:], ones_sb[:],
                                 mybir.ActivationFunctionType.Exp)

            fbT_sb = cpool.tile([D, B], BF16, tag="fbT")
            nc.sync.dma_start(fbT_sb[:], fbT[:])
            fTs_sb = cpool.tile([D, N_SLOT * 128], BF16, tag="fTs")
            Wp_sb = cpool.tile([128, NPAIR, 2, 112], FP8E4, tag="Wp")
            nc.sync.dma_start(fTs_sb[:, 0:2 * 128], fTs[:, 0:2 * 128])
            nc.sync.dma_start(Wp_sb[:, 0, :, :], Wp[:, 0:224])
            m33_sb = cpool.tile([128, 2, 512], BF16, tag="m33")
            ohb1_sb = cpool.tile([128, B], BF16, tag="ohb1")
            ad_sb = cpool.tile([128, 128], BF16, tag="ad")
            W32_sb = cpool.tile([128, 101], FP8E4, tag="W32")
            Wsup_sb = cpool.tile([128, 101], BF16, tag="Wsup")
            esup_sb = cpool.tile([128, B], BF16, tag="esup")
            ohb2_sb = cpool.tile([128, B], BF16, tag="ohb2")
            FCH = 8
            first = True
            for s0 in range(2, N_SLOT, FCH):
                s1 = min(s0 + FCH, N_SLOT)
                nc.sync.dma_start(fTs_sb[:, s0 * 128:s1 * 128],
                                  fTs[:, s0 * 128:s1 * 128])
                # pair ip occupies positions (3+2*ip, 4+2*ip); block ip+1
                p0, p1 = max((s0 - 1) // 2, 1), min((s1 - 1) // 2, NPAIR)
                if p0 < p1:
                    nc.sync.dma_start(Wp_sb[:, p0:p1, :, :],
                                      Wp[:, p0 * 224:p1 * 224])
                if first:
                    nc.sync.dma_start(m33_sb[:], m33[:])
                    nc.sync.dma_start(ad_sb[:], antidiag[:])
                    nc.sync.dma_start(W32_sb[:], W32[:])
                    nc.sync.dma_start(ohb1_sb[:], ohb1[:])
                    nc.sync.dma_start(Wsup_sb[:], Wsup[:])
                    nc.sync.dma_start(esup_sb[:], esup[:])
                    first = False
            nc.sync.dma_start(ohb2_sb[:], ohb2[:])

            # PE p-state ramp warm-up (overlaps the DMA head)
            for _ in range(4):
                Pw = mps.tile([128, 2, 512], F32, tag="P")
                nc.tensor.matmul(Pw[:, 0, :], warm_sb[:, 0:128], warm_sb[:],
                                 start=True, stop=True)

            acc2 = aps.tile([101, B], F32, tag="acc2")   # 2 banks

            def pass1(slot_id):
                P = mps.tile([128, 2, 512], F32, tag="P")
                blk = pos_of[slot_id]
                for h in range(2):
                    nc.tensor.matmul(
                        P[:, h, :],
                        fTs_sb[:, blk * 128:(blk + 1) * 128],
                        fbT_sb[:, h * 512:(h + 1) * 512],
                        start=True, stop=True,
                    )
                return P

            def exp_to(slot_id, Edst, P):
                """Edst: [128, 2, 512] e5m2 view for this slot."""
                if slot_id in ACT_SLOTS:
                    nc.scalar.activation(Edst, P[:, :, :],
                                         mybir.ActivationFunctionType.Exp,
                                         bias=bias_sb[:])
                else:
                    nc.vector.tensor_scalar(
                        Edst.bitcast(I8), P[:, :, :], SCH_A5, SCH_B5,
                        op0=mybir.AluOpType.mult, op1=mybir.AluOpType.add,
                    )

            def masks(slot_id, Ep, sub):
                if slot_id < 8:      # b2 diag block (core 0 data; ones elsewhere)
                    o = slot_id * 128
                    h, oc = o // 512, o % 512
                    nc.gpsimd.tensor_tensor(
                        Ep[:, h, sub, oc:oc + 128], Ep[:, h, sub, oc:oc + 128],
                        ad_sb[:], op=mybir.AluOpType.mult,
                    )
                if slot_id == 33:    # b1 diag mask
                    nc.gpsimd.tensor_tensor(
                        Ep[:, :, sub, :], Ep[:, :, sub, :], m33_sb[:],
                        op=mybir.AluOpType.mult,
                    )

            # ---- branch-1 pass1/exp first; its DR + select interleave
            # with the first main pairs (P-pool rotation frees naturally) ----
            Eb1 = epool.tile([128, 2, 2, 512], FP8E5, tag="Ep")
            Pb1 = [pass1(sid) for sid in ACC1_PAIR]
            for sub, sid in enumerate(ACC1_PAIR):
                exp_to(sid, Eb1[:, :, sub, :], Pb1[sub])
                masks(sid, Eb1, sub)
            partS_sb = spool.tile([128, 16], F32, tag="pS")

            pending = []   # (ip, Ep) pairs whose DR is deferred

            def flush_dr(last=False):
                ip, Ep = pending.pop(0)
                for h in range(2):
                    nc.tensor.matmul(
                        acc2[:, h * 512:(h + 1) * 512],
                        Wp_sb[:, ip + 1, :, 0:101],
                        Ep[:, h, :, :],
                        start=False, stop=(last and not pending),
                        perf_mode=mybir.MatmulPerfMode.DoubleRow,
                    )

            def do_pair(ip):
                sa, sb = PAIRS[ip]
                Ep = epool.tile([128, 2, 2, 512], FP8E5, tag="Ep")
                for sub, sid in enumerate((sa, sb)):
                    P = pass1(sid)
                    exp_to(sid, Ep[:, :, sub, :], P)
                    masks(sid, Ep, sub)
                pending.append((ip, Ep))
                # flush the DR of the pair emitted two earlier: by now its
                # exps are done, so the PE never stalls on a committed LDW.
                # Near the end, drain to lag 1 so the tail holds fewer DRs.
                if len(pending) > 4:
                    flush_dr()

            # acc2 groups open on the first pair's DR (start=True there)
            do_pair(0)
            do_pair(1)
            do_pair(2)

            # branch-1 DR + sup-logits block (emitted late so the greedy
            # PE scheduler keeps preferring the pair pipeline; allocating
            # acc1 here keeps its P-pool buffer pinned only briefly)
            acc1 = mps.tile([101, B], F32, tag="P")
            for h in range(2):
                nc.tensor.matmul(
                    acc1[:, h * 512:(h + 1) * 512],
                    Wp_sb[:, 0, :, 0:101],
                    Eb1[:, h, :, :],
                    start=True, stop=True,
                    perf_mode=mybir.MatmulPerfMode.DoubleRow,
                )
            for h in range(2):
                nc.tensor.matmul(
                    acc2[:, h * 512:(h + 1) * 512],
                    Wsup_sb[:],
                    esup_sb[:, h * 512:(h + 1) * 512],
                    start=True, stop=False,
                )

            # leftover single slot (plain fp8 matmuls)
            E32 = epool.tile([128, 2, 2, 512], FP8E5, tag="Ep")
            P = pass1(SINGLE)
            exp_to(SINGLE, E32[:, :, 0, :], P)
            for h in range(2):
                nc.tensor.matmul(
                    acc2[:, h * 512:(h + 1) * 512],
                    W32_sb[:],
                    E32[:, h, 0, :],
                    start=False, stop=False,
                )

            # branch-1 select: frees acc1's P-pool buffer for the pipeline
            sel1 = spool.tile([128, B], F32, tag="sel")
            nc.vector.tensor_tensor(sel1[:101, :], acc1[0:101, :],
                                    ohb1_sb[:101, :], op=mybir.AluOpType.mult)
            Sp1 = mps.tile([128, 8], F32, tag="P")
            for c in range(8):
                nc.tensor.matmul(
                    Sp1[:, c:c + 1],
                    sel1[:101, c * 128:(c + 1) * 128],
                    ones_sb[:101, :],
                    start=True, stop=True,
                )
            nc.vector.tensor_copy(partS_sb[:, 8:16], Sp1[:])
            nc.sync.dma_start(partS[:, 8:16], partS_sb[:, 8:16])

            for ip in range(3, len(PAIRS)):
                do_pair(ip)
            while pending:
                flush_dr(last=True)

            # ---- S2 select (both banks close within ~200ns, so one op) ----
            sel2 = spool.tile([128, B], F32, tag="sel")
            Sp2 = mps.tile([128, 8], F32, tag="P")
            nc.vector.tensor_tensor(sel2[:101, :], acc2[0:101, :],
                                    ohb2_sb[:101, :], op=mybir.AluOpType.mult)
            for c in range(8):
                nc.tensor.matmul(
                    Sp2[:, c:c + 1],
                    sel2[:101, c * 128:(c + 1) * 128],
                    ones_sb[:101, :],
                    start=True, stop=True,
                )
            nc.vector.tensor_copy(partS_sb[:, 0:8], Sp2[:])
            nc.sync.dma_start(partS[:, 0:8], partS_sb[:, 0:8])

    nc.compile()
    return nc


def _prep_inputs(features, sup_logits, centers, labels):
    f = features.astype(np.float32)
    lab = labels.astype(np.int64)
    labB = lab[:B]
    ccount = np.bincount(lab, minlength=C).astype(np.float64)
    cntB = np.bincount(labB, minlength=C).astype(np.float64)
    cc1 = cntB + 1.0

    fbT = np.ascontiguousarray((f[:B] / T).T).astype(NP_BF16)          # [D, B]
    fT = f.T                                                           # [D, N]

    r0 = (1.0 / ccount[lab]).astype(np.float64)
    dv = 1.0 / (ccount[lab] - ALPHA) - r0
    lab1 = np.concatenate([labB, np.arange(C)])
    r0_1 = 1.0 / cc1[lab1]
    den1 = cc1[lab1] - 1.0
    d1 = np.where(den1 > 0, 1.0 / np.maximum(den1, 1e-30) - r0_1, 0.0)

    esupT = np.zeros((128, B), np.float64)
    esupT[:C, :] = np.exp(sup_logits.astype(np.float64)).T * 2.0 ** -5

    eye = np.eye(128, dtype=np.float32)
    # select masks fold the per-row fp8 scale compensation
    ohb2 = np.zeros((128, B), np.float64)
    ohb2[0, :] = 2.0 ** -7
    ohb2[1 + labB, np.arange(B)] = 2.0 ** -20
    ohb1v = np.zeros((128, B), np.float64)
    ohb1v[0, :] = 2.0 ** -3
    ohb1v[1 + labB, np.arange(B)] = 2.0 ** -4

    def w_slot(col_lab, r0v, dvv, npart, s_lane, base_exp0, base_exp1):
        w = np.zeros((128, 101), np.float64)
        w[:npart, 0] = r0v * 2.0 ** (base_exp0 + s_lane)
        w[np.arange(npart), 1 + col_lab] = dvv * 2.0 ** (base_exp1 + s_lane)
        return np.clip(w, 0.0, 240.0)

    def lane_s(slot_id):
        return 5 if slot_id in ACT_SLOTS else 2

    in_maps = []
    for c in range(M):
        cols = np.zeros((D, N_SLOT * 128), np.float32)
        Wpc = np.zeros((128, NPAIR * 224), np.float64)

        def slot_w(slot_id):
            if slot_id < JC_B2:
                j0 = c * SHARD + slot_id * 128
                return w_slot(lab[j0:j0 + 128], r0[j0:j0 + 128],
                              dv[j0:j0 + 128], 128, lane_s(slot_id), 7, 20)
            if slot_id == 33:   # b1 batch chunk
                ch = slice(c * 128, (c + 1) * 128)
                return w_slot(labB[ch], r0_1[ch.start:ch.stop],
                              d1[ch.start:ch.stop], 128, lane_s(33), 3, 4)
            # slot34: centers (core 0 only)
            if c == 0:
                return w_slot(np.arange(C), r0_1[B:], d1[B:], C, lane_s(34), 3, 4)
            return np.zeros((128, 101), np.float64)

        # fTs in processing order
        for pos, sid in enumerate(PROC):
            if sid < JC_B2:
                sl = slice(c * SHARD + sid * 128, c * SHARD + (sid + 1) * 128)
                cols[:, pos * 128:(pos + 1) * 128] = fT[:, sl]
            elif sid == 33:
                cols[:, pos * 128:(pos + 1) * 128] = fT[:, c * 128:(c + 1) * 128]
            elif sid == 34 and c == 0:
                cols[:, pos * 128:pos * 128 + C] = centers.T
        # W pair blocks: block 0 = acc1 pair, 1.. = acc2 pairs
        for bi, (sa, sb) in enumerate([ACC1_PAIR] + PAIRS):
            Wpc[:, bi * 224 + 0:bi * 224 + 101] = slot_w(sa)
            Wpc[:, bi * 224 + 112:bi * 224 + 213] = slot_w(sb)
        W32c = slot_w(SINGLE)

        Wsupc = np.zeros((128, 101), np.float64)
        if c == 0:
            Wsupc[:C, 0] = (1.0 / ccount) * 2.0 ** 12
            Wsupc[np.arange(C), 1 + np.arange(C)] = (
                1.0 / (ccount - 1.0) - 1.0 / ccount) * 2.0 ** 25
            esup_c = esupT
            ad = 1.0 - eye
        else:
            esup_c = np.zeros((128, B), np.float64)
            ad = np.ones((128, 128), np.float32)
        m33c = np.ones((128, B), np.float64)
        m33c[:, c * 128:(c + 1) * 128] = 1.0 - eye

        in_maps.append({
            "fbT": fbT,
            "fTs": cols.astype(NP_BF16),
            "Wp": Wpc.astype(NP_FP8E4),
            "W32": W32c.astype(NP_FP8E4),
            "Wsup": Wsupc.astype(NP_BF16),
            "antidiag": ad.astype(NP_BF16),
            "m33": m33c.astype(NP_BF16),
            "ohb1": ohb1v.astype(NP_BF16),
            "ohb2": ohb2.astype(NP_BF16),
            "esup": esup_c.astype(NP_BF16),
        })
    return in_maps


def kernel(features, sup_logits, centers, labels, _debug=False, _trace=False):
    if "nc" not in _CACHE:
        _CACHE["nc"] = _build_nc()
    nc = _CACHE["nc"]
    in_maps = _prep_inputs(features, sup_logits, centers, labels)
    res = run_bass_kernel_spmd(nc, in_maps, core_ids=list(range(M)))
    _CACHE["last"] = res

    f = features.astype(np.float64)
    lab = labels.astype(np.int64)
    labB = lab[:B]
    ccount = np.bincount(lab, minlength=C).astype(np.float64)
    cntB = np.bincount(labB, minlength=C).astype(np.float64)

    S2 = np.zeros(B, np.float64)
    S1 = np.zeros(B, np.float64)
    for c in range(M):
        pS = res.results[c]["partS"].astype(np.float64)
        S2 += pS[:, 0:8].T.reshape(B)
        S1 += pS[:, 8:16].T.reshape(B)

    # A vectors on host: A_i = f_i . g_{lab_i} / T  minus the diag term
    g2 = np.zeros((C, D), np.float64)
    np.add.at(g2, lab, f)
    g1 = np.zeros((C, D), np.float64)
    np.add.at(g1, labB, f[:B])
    g1 += centers.astype(np.float64)
    A2 = np.einsum("bd,bd->b", f[:B], g2[labB]) / T - 1.0 / T
    A1 = np.einsum("bd,bd->b", f[:B], g1[labB]) / T - 1.0 / T

    N2 = ccount[labB] - 1.0
    msum = 1.0 + ALPHA * N2
    numer2 = sup_logits.astype(np.float64)[np.arange(B), labB] + ALPHA * A2
    loss2 = np.mean(np.log(S2) - numer2 / msum)
    N1 = cntB[labB]
    loss1 = np.mean(np.log(S1) - A1 / N1)
    return np.array(loss1 + loss2, dtype=np.float32)
